# revision 1
# baseline (speedup 1.0000x reference)
"""Causal self-attention on 8 Trainium2 NeuronCores.

Problem (hardcoded): B=4, T=2048, C=1024, H=16, D=64.
  qkv = x @ w_qkv + b_qkv ; per-head causal softmax attention ; out = attn @ w_proj + b_proj

Sharding (per hint): tensor-parallel over heads x data-parallel over batch.
  core c -> batch b = c // 2, head group g = c % 2 (heads g*8 .. g*8+7).
Each core computes QKV for its 8 heads, causal attention, and a partial
projection (its 512 input channels of w_proj). Host sums the two partials per
batch and adds b_proj.

On-core layout ("transposed" attention so softmax reduction lands on the
matmul contraction axis):
  xT   [C, T]  (host pre-transposed, bf16)
  QT,KT [d, t] per head, 2 heads stacked per 128 partitions
  V_aug [t, 65] per head (col 64 = ones -> PV matmul emits softmax denom)
  S^T  [j, i] tiles from lhsT=KT, rhs=QT (K=64 contraction, head-pair packed
       into array row-halves 0-63 / 64-127 for 2x concurrency)
  P = exp(S^T/8) (ScalarE, PSUM->SBUF bf16), causal diag masked by
       precomputed 0/1 tiles (VectorE mul); off-diagonal j>i tiles skipped.
  O_aug^T [65, i] accumulated over j chunks; row 64 = denominator.
  AT = O^T * (1/denom) broadcast -> proj lhsT; partial = A @ w_proj_slice.
"""

import numpy as np
import ml_dtypes

B, T, C, H, D = 4, 2048, 1024, 16, 64
HL = H // 2          # heads per core
CL = HL * D          # local channels (512)
NPAIR = HL // 2      # head pairs per core (4)
CCH = C // 128       # contraction chunks for qkv (8)
PCH = CL // 128      # contraction chunks for proj (4)
TT = T // 128        # t tiles (16)
NI = T // 512        # i chunks (4)
N_CORES = 8
BF16 = ml_dtypes.bfloat16

_compiled = None  # (nc,) cache


def _build(nc):
    import concourse.tile as tile
    from concourse import mybir

    bf = mybir.dt.bfloat16
    f32 = mybir.dt.float32
    Exp = mybir.ActivationFunctionType.Exp

    xT = nc.dram_tensor("xT", [C, T], bf, kind="ExternalInput").ap()
    wq = nc.dram_tensor("wq", [C, CL], bf, kind="ExternalInput").ap()
    wk = nc.dram_tensor("wk", [C, CL], bf, kind="ExternalInput").ap()
    wv = nc.dram_tensor("wv", [C, CL], bf, kind="ExternalInput").ap()
    bq = nc.dram_tensor("bq", [128, NPAIR], f32, kind="ExternalInput").ap()
    bk = nc.dram_tensor("bk", [128, NPAIR], f32, kind="ExternalInput").ap()
    bv = nc.dram_tensor("bv", [128, CL], f32, kind="ExternalInput").ap()
    wp = nc.dram_tensor("wp", [CL, C], bf, kind="ExternalInput").ap()
    out = nc.dram_tensor("out", [T, C], f32, kind="ExternalOutput").ap()

    xT_r = xT.rearrange("(cc p) t -> p cc t", p=128)
    wq_r = wq.rearrange("(cc p) m -> p cc m", p=128)
    wk_r = wk.rearrange("(cc p) m -> p cc m", p=128)
    wv_r = wv.rearrange("(cc p) m -> p cc m", p=128)
    wp_r = wp.rearrange("(cc p) n -> p cc n", p=128)

    with tile.TileContext(nc) as tc:
        import contextlib

        with contextlib.ExitStack() as ctx:
            persist = ctx.enter_context(tc.tile_pool(name="persist", bufs=1))
            mm_ps = ctx.enter_context(tc.tile_pool(name="mm_ps", bufs=2, space="PSUM"))
            s_ps = ctx.enter_context(tc.tile_pool(name="s_ps", bufs=2, space="PSUM"))
            o_ps = ctx.enter_context(tc.tile_pool(name="o_ps", bufs=1, space="PSUM"))
            p_pool = ctx.enter_context(tc.tile_pool(name="p_pool", bufs=3))
            r_pool = ctx.enter_context(tc.tile_pool(name="r_pool", bufs=2))
            st_pool = ctx.enter_context(tc.tile_pool(name="st_pool", bufs=3))

            # ---- persistent SBUF tensors ----
            xT_sb = persist.tile([128, CCH, T], bf)
            wq_sb = persist.tile([128, CCH, CL], bf)
            wk_sb = persist.tile([128, CCH, CL], bf)
            wv_sb = persist.tile([128, CCH, CL], bf)
            wp_sb = persist.tile([128, PCH, C], bf)
            bq_sb = persist.tile([128, NPAIR], f32)
            bk_sb = persist.tile([128, NPAIR], f32)
            bv_sb = persist.tile([128, CL], f32)
            QT_sb = persist.tile([128, NPAIR, T], bf)
            KT_sb = persist.tile([128, NPAIR, T], bf)
            V_sb = persist.tile([128, TT, HL, D + 1], bf)
            AT_sb = persist.tile([128, PCH, T], bf)
            m_sb = persist.tile([128, 4, 512], bf)

            for cc in range(CCH):
                nc.sync.dma_start(out=xT_sb[:, cc, :], in_=xT_r[:, cc, :])
                nc.sync.dma_start(out=wq_sb[:, cc, :], in_=wq_r[:, cc, :])
                nc.sync.dma_start(out=wk_sb[:, cc, :], in_=wk_r[:, cc, :])
                nc.sync.dma_start(out=wv_sb[:, cc, :], in_=wv_r[:, cc, :])
            for cc in range(PCH):
                nc.sync.dma_start(out=wp_sb[:, cc, :], in_=wp_r[:, cc, :])
            nc.sync.dma_start(out=bq_sb[:], in_=bq[:])
            nc.sync.dma_start(out=bk_sb[:], in_=bk[:])
            nc.sync.dma_start(out=bv_sb[:], in_=bv[:])

            # causal 0/1 masks for the 4 diagonal j-tile offsets:
            # m[r][jj, ii] = 1 if ii - jj >= 128*r else 0
            for r in range(4):
                nc.vector.memset(m_sb[:, r, :], 1.0)
                nc.gpsimd.affine_select(
                    out=m_sb[:, r, :],
                    in_=m_sb[:, r, :],
                    compare_op=mybir.AluOpType.is_ge,
                    fill=0.0,
                    base=-128 * r,
                    pattern=[[1, 512]],
                    channel_multiplier=-1,
                )
            # ones column of V_aug
            nc.vector.memset(V_sb[:, :, :, D], 1.0)

            # ---- QKV projection ----
            for pair in range(NPAIR):
                for ti in range(T // 512):
                    for w_sb, dst, b_sb in (
                        (wq_sb, QT_sb, bq_sb),
                        (wk_sb, KT_sb, bk_sb),
                    ):
                        ps = mm_ps.tile([128, 512], f32, tag="mmps")
                        for cc in range(CCH):
                            nc.tensor.matmul(
                                ps[:],
                                lhsT=w_sb[:, cc, pair * 128 : (pair + 1) * 128],
                                rhs=xT_sb[:, cc, ti * 512 : (ti + 1) * 512],
                                start=(cc == 0),
                                stop=(cc == CCH - 1),
                            )
                        nc.vector.tensor_scalar_add(
                            dst[:, pair, ti * 512 : (ti + 1) * 512],
                            ps[:],
                            b_sb[:, pair : pair + 1],
                        )
            for tt in range(TT):
                ps = mm_ps.tile([128, 512], f32, tag="mmps")
                for cc in range(CCH):
                    nc.tensor.matmul(
                        ps[:],
                        lhsT=xT_sb[:, cc, tt * 128 : (tt + 1) * 128],
                        rhs=wv_sb[:, cc, :],
                        start=(cc == 0),
                        stop=(cc == CCH - 1),
                    )
                nc.vector.tensor_add(
                    V_sb[:, tt, :, 0:D],
                    ps[:].rearrange("p (h d) -> p h d", h=HL),
                    bv_sb[:].rearrange("p (h d) -> p h d", h=HL),
                )

            # ---- attention ----
            for ci in range(NI):
                for pair in range(NPAIR):
                    o0 = o_ps.tile([D + 1, 512], f32, tag="o0")
                    o1 = o_ps.tile([D + 1, 512], f32, tag="o1")
                    njt = 4 * (ci + 1)
                    for jt in range(njt):
                        ptiles = []
                        for s in range(2):
                            st = s_ps.tile([128, 512], f32, tag=f"s{s}")
                            nc.tensor.matmul(
                                st[:],
                                lhsT=KT_sb[
                                    64 * s : 64 * (s + 1),
                                    pair,
                                    jt * 128 : (jt + 1) * 128,
                                ],
                                rhs=QT_sb[
                                    64 * s : 64 * (s + 1),
                                    pair,
                                    ci * 512 : (ci + 1) * 512,
                                ],
                                start=True,
                                stop=True,
                            )
                            pt = p_pool.tile([128, 512], bf, tag=f"p{s}")
                            nc.scalar.activation(pt[:], st[:], Exp, scale=0.125)
                            r = jt - 4 * ci
                            if r >= 0:
                                nc.vector.tensor_mul(pt[:], pt[:], m_sb[:, r, :])
                            ptiles.append(pt)
                        for s, (ot, pt) in enumerate(zip((o0, o1), ptiles)):
                            nc.tensor.matmul(
                                ot[:],
                                lhsT=V_sb[:, jt, 2 * pair + s, :],
                                rhs=pt[:],
                                start=(jt == 0),
                                stop=(jt == njt - 1),
                            )
                    for s, ot in enumerate((o0, o1)):
                        rc = r_pool.tile([1, 512], f32, tag="rc")
                        nc.vector.reciprocal(rc[:], ot[D : D + 1, :])
                        rb = r_pool.tile([128, 512], f32, tag="rb")
                        nc.gpsimd.partition_broadcast(rb[:], rc[:])
                        nc.vector.tensor_mul(
                            AT_sb[
                                64 * s : 64 * (s + 1),
                                pair,
                                ci * 512 : (ci + 1) * 512,
                            ],
                            ot[0:D, :],
                            rb[64 * s : 64 * (s + 1), :],
                        )

            # ---- partial projection ----
            for tt in range(TT):
                for nh in range(C // 512):
                    ps = mm_ps.tile([128, 512], f32, tag="mmps")
                    for cc in range(PCH):
                        nc.tensor.matmul(
                            ps[:],
                            lhsT=AT_sb[:, cc, tt * 128 : (tt + 1) * 128],
                            rhs=wp_sb[:, cc, nh * 512 : (nh + 1) * 512],
                            start=(cc == 0),
                            stop=(cc == PCH - 1),
                        )
                    so = st_pool.tile([128, 512], f32, tag="so")
                    nc.scalar.copy(so[:], ps[:])
                    nc.sync.dma_start(
                        out=out[tt * 128 : (tt + 1) * 128, nh * 512 : (nh + 1) * 512],
                        in_=so[:],
                    )
    return nc


def _get_compiled():
    global _compiled
    if _compiled is None:
        from concourse import bacc

        nc = bacc.Bacc(
            "TRN2", target_bir_lowering=False, debug=False, num_devices=N_CORES
        )
        _build(nc)
        nc.compile()
        _compiled = nc
    return _compiled


def _shard_inputs(x, w_qkv, b_qkv, w_proj):
    """Build the 8 per-core input dicts (host-side transpose/slice/cast)."""
    in_maps = []
    wq_f, wk_f, wv_f = w_qkv[:, :C], w_qkv[:, C : 2 * C], w_qkv[:, 2 * C :]
    for c in range(N_CORES):
        b, g = c // 2, c % 2
        sl = slice(g * CL, (g + 1) * CL)
        bqs = np.ascontiguousarray(b_qkv[0 * C :][sl].reshape(NPAIR, 128).T)
        bks = np.ascontiguousarray(b_qkv[1 * C :][sl].reshape(NPAIR, 128).T)
        bvs = np.ascontiguousarray(
            np.broadcast_to(b_qkv[2 * C :][sl][None, :], (128, CL))
        )
        in_maps.append(
            {
                "xT": np.ascontiguousarray(x[b].T).astype(BF16),
                "wq": np.ascontiguousarray(wq_f[:, sl]).astype(BF16),
                "wk": np.ascontiguousarray(wk_f[:, sl]).astype(BF16),
                "wv": np.ascontiguousarray(wv_f[:, sl]).astype(BF16),
                "bq": bqs.astype(np.float32),
                "bk": bks.astype(np.float32),
                "bv": bvs.astype(np.float32),
                "wp": np.ascontiguousarray(w_proj[sl, :]).astype(BF16),
            }
        )
    return in_maps


def kernel(x, w_qkv, b_qkv, w_proj, b_proj, _trace=False, _tmpdir=None):
    from concourse.bass_utils import run_bass_kernel_spmd

    x = np.asarray(x, dtype=np.float32)
    w_qkv = np.asarray(w_qkv, dtype=np.float32)
    b_qkv = np.asarray(b_qkv, dtype=np.float32)
    w_proj = np.asarray(w_proj, dtype=np.float32)
    b_proj = np.asarray(b_proj, dtype=np.float32)

    nc = _get_compiled()
    in_maps = _shard_inputs(x, w_qkv, b_qkv, w_proj)
    res = run_bass_kernel_spmd(
        nc,
        in_maps,
        core_ids=list(range(N_CORES)),
        trace=_trace,
        tmpdir=_tmpdir,
    )
    out = np.empty((B, T, C), dtype=np.float32)
    for b in range(B):
        out[b] = res.results[2 * b]["out"] + res.results[2 * b + 1]["out"] + b_proj
    kernel._last_result = res
    return out


# revision 2
# speedup vs baseline: 1.3312x; 1.3312x over previous
"""Causal self-attention on 8 Trainium2 NeuronCores.

Problem (hardcoded): B=4, T=2048, C=1024, H=16, D=64.
  qkv = x @ w_qkv + b_qkv ; per-head causal softmax attention ; out = attn @ w_proj + b_proj

Sharding (per hint): tensor-parallel over heads x data-parallel over batch.
  core c -> batch b = c // 2, head group g = c % 2 (heads g*8 .. g*8+7).
Each core computes QKV for its 8 heads, causal attention, and a partial
projection (its 512 input channels of w_proj). Host sums the two partials per
batch and adds b_proj.

On-core layout ("transposed" attention so softmax reduction lands on the
matmul contraction axis):
  xT   [C, T]  (host pre-transposed, bf16)
  QT,KT [d, t] per head, 2 heads stacked per 128 partitions
  V_aug [t, 65] per head (col 64 = ones -> PV matmul emits softmax denom)
  S^T  [j, i] tiles from lhsT=KT, rhs=QT (K=64 contraction); the head pair's
       two S tiles land in one [128, 2, 512] PSUM tile (2 banks) so a single
       Exp activation serves both heads.
  P = exp(S^T/8) (ScalarE, PSUM->SBUF bf16); causal diagonal tiles masked by
       a precomputed 0/1 multiply (VectorE); off-diagonal j>i tiles skipped.
  O_aug^T [65, i] accumulated over j chunks per head; row 64 = denominator.
  AT = O^T * (1/denom) broadcast -> proj lhsT; partial = A @ w_proj_slice.
"""

import numpy as np
import ml_dtypes

B, T, C, H, D = 4, 2048, 1024, 16, 64
HL = H // 2          # heads per core
CL = HL * D          # local channels (512)
NPAIR = HL // 2      # head pairs per core (4)
CCH = C // 128       # contraction chunks for qkv (8)
PCH = CL // 128      # contraction chunks for proj (4)
TT = T // 128        # t tiles (16)
NI = T // 512        # i chunks (4)
N_CORES = 8
BF16 = ml_dtypes.bfloat16

_compiled = None


def _build(nc):
    import concourse.tile as tile
    from concourse import mybir

    bf = mybir.dt.bfloat16
    f32 = mybir.dt.float32
    Exp = mybir.ActivationFunctionType.Exp

    xT = nc.dram_tensor("xT", [C, T], bf, kind="ExternalInput").ap()
    wq = nc.dram_tensor("wq", [C, CL], bf, kind="ExternalInput").ap()
    wk = nc.dram_tensor("wk", [C, CL], bf, kind="ExternalInput").ap()
    wv = nc.dram_tensor("wv", [C, CL], bf, kind="ExternalInput").ap()
    bq = nc.dram_tensor("bq", [128, NPAIR], f32, kind="ExternalInput").ap()
    bk = nc.dram_tensor("bk", [128, NPAIR], f32, kind="ExternalInput").ap()
    bv = nc.dram_tensor("bv", [128, CL], f32, kind="ExternalInput").ap()
    wp = nc.dram_tensor("wp", [CL, C], bf, kind="ExternalInput").ap()
    out = nc.dram_tensor("out", [T, C], f32, kind="ExternalOutput").ap()

    xT_r = xT.rearrange("(cc p) t -> p cc t", p=128)
    wq_r = wq.rearrange("(cc p) m -> p cc m", p=128)
    wk_r = wk.rearrange("(cc p) m -> p cc m", p=128)
    wv_r = wv.rearrange("(cc p) m -> p cc m", p=128)
    wp_r = wp.rearrange("(cc p) n -> p cc n", p=128)

    with tile.TileContext(nc) as tc:
        import contextlib

        with contextlib.ExitStack() as ctx:
            persist = ctx.enter_context(tc.tile_pool(name="persist", bufs=1))
            # PSUM: tag "s" slots are [128, 2, 512] (2 banks) x 3 bufs = 6
            # banks; o0/o1 are 1 bank x 1 buf each -> 8 banks total.
            ps_pool = ctx.enter_context(tc.tile_pool(name="ps_pool", bufs=3, space="PSUM"))
            o_ps = ctx.enter_context(tc.tile_pool(name="o_ps", bufs=1, space="PSUM"))
            p_pool = ctx.enter_context(tc.tile_pool(name="p_pool", bufs=3))
            r_pool = ctx.enter_context(tc.tile_pool(name="r_pool", bufs=2))
            st_pool = ctx.enter_context(tc.tile_pool(name="st_pool", bufs=3))

            # ---- persistent SBUF tensors ----
            xT_sb = persist.tile([128, CCH, T], bf)
            wq_sb = persist.tile([128, CCH, CL], bf)
            wk_sb = persist.tile([128, CCH, CL], bf)
            wv_sb = persist.tile([128, CCH, CL], bf)
            wp_sb = persist.tile([128, PCH, C], bf)
            bq_sb = persist.tile([128, NPAIR], f32)
            bk_sb = persist.tile([128, NPAIR], f32)
            bv_sb = persist.tile([128, CL], f32)
            QT_sb = persist.tile([128, NPAIR, T], bf)
            KT_sb = persist.tile([128, NPAIR, T], bf)
            V_sb = persist.tile([128, TT, HL, D + 1], bf)
            AT_sb = persist.tile([128, PCH, T], bf)
            m_sb = persist.tile([128, 4, 2, 512], bf)

            for cc in range(CCH):
                nc.sync.dma_start(out=xT_sb[:, cc, :], in_=xT_r[:, cc, :])
                nc.sync.dma_start(out=wq_sb[:, cc, :], in_=wq_r[:, cc, :])
                nc.sync.dma_start(out=wk_sb[:, cc, :], in_=wk_r[:, cc, :])
                nc.sync.dma_start(out=wv_sb[:, cc, :], in_=wv_r[:, cc, :])
            for cc in range(PCH):
                nc.sync.dma_start(out=wp_sb[:, cc, :], in_=wp_r[:, cc, :])
            nc.sync.dma_start(out=bq_sb[:], in_=bq[:])
            nc.sync.dma_start(out=bk_sb[:], in_=bk[:])
            nc.sync.dma_start(out=bv_sb[:], in_=bv[:])

            # causal 0/1 masks, replicated for the pair dim:
            # m[r][jj, :, ii] = 1 if ii - jj >= 128*r else 0
            for r in range(4):
                nc.vector.memset(m_sb[:, r], 1.0)
                nc.gpsimd.affine_select(
                    out=m_sb[:, r],
                    in_=m_sb[:, r],
                    compare_op=mybir.AluOpType.is_ge,
                    fill=0.0,
                    base=-128 * r,
                    pattern=[[0, 2], [1, 512]],
                    channel_multiplier=-1,
                )
            # ones column of V_aug
            nc.vector.memset(V_sb[:, :, :, D], 1.0)

            # ---- QKV projection ----
            for pair in range(NPAIR):
                for th in range(T // 1024):
                    for w_sb, dst, b_sb in (
                        (wq_sb, QT_sb, bq_sb),
                        (wk_sb, KT_sb, bk_sb),
                    ):
                        ps = ps_pool.tile([128, 2, 512], f32, tag="s")
                        for h2 in range(2):
                            t0 = th * 1024 + h2 * 512
                            for cc in range(CCH):
                                nc.tensor.matmul(
                                    ps[:, h2, :],
                                    lhsT=w_sb[:, cc, pair * 128 : (pair + 1) * 128],
                                    rhs=xT_sb[:, cc, t0 : t0 + 512],
                                    start=(cc == 0),
                                    stop=(cc == CCH - 1),
                                )
                        nc.vector.tensor_scalar_add(
                            dst[:, pair, th * 1024 : (th + 1) * 1024],
                            ps[:].rearrange("p a b -> p (a b)"),
                            b_sb[:, pair : pair + 1],
                        )
            for tt2 in range(TT // 2):
                ps = ps_pool.tile([128, 2, 512], f32, tag="s")
                for h2 in range(2):
                    tt = tt2 * 2 + h2
                    for cc in range(CCH):
                        nc.tensor.matmul(
                            ps[:, h2, :],
                            lhsT=xT_sb[:, cc, tt * 128 : (tt + 1) * 128],
                            rhs=wv_sb[:, cc, :],
                            start=(cc == 0),
                            stop=(cc == CCH - 1),
                        )
                    nc.vector.tensor_add(
                        V_sb[:, tt, :, 0:D],
                        ps[:, h2, :].rearrange("p (h d) -> p h d", h=HL),
                        bv_sb[:].rearrange("p (h d) -> p h d", h=HL),
                    )

            # ---- attention ----
            for ci in range(NI):
                for pair in range(NPAIR):
                    o0 = o_ps.tile([D + 1, 512], f32, tag="o0")
                    o1 = o_ps.tile([D + 1, 512], f32, tag="o1")
                    njt = 4 * (ci + 1)
                    for jt in range(njt):
                        st = ps_pool.tile([128, 2, 512], f32, tag="s")
                        for s in range(2):
                            nc.tensor.matmul(
                                st[:, s, :],
                                lhsT=KT_sb[
                                    64 * s : 64 * (s + 1),
                                    pair,
                                    jt * 128 : (jt + 1) * 128,
                                ],
                                rhs=QT_sb[
                                    64 * s : 64 * (s + 1),
                                    pair,
                                    ci * 512 : (ci + 1) * 512,
                                ],
                                start=True,
                                stop=True,
                            )
                        pt = p_pool.tile([128, 2, 512], bf, tag="p")
                        nc.scalar.activation(pt[:], st[:], Exp, scale=0.125)
                        r = jt - 4 * ci
                        if r >= 0:
                            nc.vector.tensor_mul(pt[:], pt[:], m_sb[:, r])
                        for s, ot in enumerate((o0, o1)):
                            nc.tensor.matmul(
                                ot[:],
                                lhsT=V_sb[:, jt, 2 * pair + s, :],
                                rhs=pt[:, s, :],
                                start=(jt == 0),
                                stop=(jt == njt - 1),
                            )
                    for s, ot in enumerate((o0, o1)):
                        rc = r_pool.tile([1, 512], f32, tag="rc")
                        nc.vector.reciprocal_approx_fast(rc[:], ot[D : D + 1, :])
                        rb = r_pool.tile([128, 512], f32, tag="rb")
                        nc.gpsimd.partition_broadcast(rb[:], rc[:])
                        nc.vector.tensor_mul(
                            AT_sb[
                                64 * s : 64 * (s + 1),
                                pair,
                                ci * 512 : (ci + 1) * 512,
                            ],
                            ot[0:D, :],
                            rb[64 * s : 64 * (s + 1), :],
                        )

            # ---- partial projection ----
            for tt in range(TT):
                for nh in range(C // 512):
                    ps = ps_pool.tile([128, 512], f32, tag="s")
                    for cc in range(PCH):
                        nc.tensor.matmul(
                            ps[:],
                            lhsT=AT_sb[:, cc, tt * 128 : (tt + 1) * 128],
                            rhs=wp_sb[:, cc, nh * 512 : (nh + 1) * 512],
                            start=(cc == 0),
                            stop=(cc == PCH - 1),
                        )
                    so = st_pool.tile([128, 512], f32, tag="so")
                    nc.scalar.copy(so[:], ps[:])
                    nc.sync.dma_start(
                        out=out[tt * 128 : (tt + 1) * 128, nh * 512 : (nh + 1) * 512],
                        in_=so[:],
                    )
    return nc


def _get_compiled():
    global _compiled
    if _compiled is None:
        from concourse import bacc

        nc = bacc.Bacc(
            "TRN2", target_bir_lowering=False, debug=False, num_devices=N_CORES
        )
        _build(nc)
        nc.compile()
        _compiled = nc
    return _compiled


def _shard_inputs(x, w_qkv, b_qkv, w_proj):
    """Build the 8 per-core input dicts (host-side transpose/slice/cast)."""
    in_maps = []
    wq_f, wk_f, wv_f = w_qkv[:, :C], w_qkv[:, C : 2 * C], w_qkv[:, 2 * C :]
    for c in range(N_CORES):
        b, g = c // 2, c % 2
        sl = slice(g * CL, (g + 1) * CL)
        bqs = np.ascontiguousarray(b_qkv[0 * C :][sl].reshape(NPAIR, 128).T)
        bks = np.ascontiguousarray(b_qkv[1 * C :][sl].reshape(NPAIR, 128).T)
        bvs = np.ascontiguousarray(
            np.broadcast_to(b_qkv[2 * C :][sl][None, :], (128, CL))
        )
        in_maps.append(
            {
                "xT": np.ascontiguousarray(x[b].T).astype(BF16),
                "wq": np.ascontiguousarray(wq_f[:, sl]).astype(BF16),
                "wk": np.ascontiguousarray(wk_f[:, sl]).astype(BF16),
                "wv": np.ascontiguousarray(wv_f[:, sl]).astype(BF16),
                "bq": bqs.astype(np.float32),
                "bk": bks.astype(np.float32),
                "bv": bvs.astype(np.float32),
                "wp": np.ascontiguousarray(w_proj[sl, :]).astype(BF16),
            }
        )
    return in_maps


def kernel(x, w_qkv, b_qkv, w_proj, b_proj, _trace=False, _tmpdir=None):
    from concourse.bass_utils import run_bass_kernel_spmd

    x = np.asarray(x, dtype=np.float32)
    w_qkv = np.asarray(w_qkv, dtype=np.float32)
    b_qkv = np.asarray(b_qkv, dtype=np.float32)
    w_proj = np.asarray(w_proj, dtype=np.float32)
    b_proj = np.asarray(b_proj, dtype=np.float32)

    nc = _get_compiled()
    in_maps = _shard_inputs(x, w_qkv, b_qkv, w_proj)
    res = run_bass_kernel_spmd(
        nc,
        in_maps,
        core_ids=list(range(N_CORES)),
        trace=_trace,
        tmpdir=_tmpdir,
    )
    out = np.empty((B, T, C), dtype=np.float32)
    for b in range(B):
        out[b] = res.results[2 * b]["out"] + res.results[2 * b + 1]["out"] + b_proj
    kernel._last_result = res
    return out
